# revision 22
# baseline (speedup 1.0000x reference)
"""CTRNN forward kernel for Trainium2 (8 NeuronCores, data-parallel over batch).

Reference computation (per step t, dt=0.02):
    h = h*(1-dt) + dt*(tanh(h) @ J.T + v_t @ Bmat.T)
    out_t = tanh(h) @ W_ro.T

Device mapping (per core, B_LOC=16 of the 128 batch rows):
  - Layout: [hidden-on-partitions x batch-on-free] ("hT"); hidden 512 =
    4 chunks of 128 partitions; chunk q row p = hidden index 128*q+p.
  - State is rescaled: PSUM holds H = h/dt in 2 half banks [128, 32]
    (bank Hf holds chunks 2Hf, 2Hf+1 at cols 0:16 / 16:32), never
    evicted:  H_{t+1} = 0.98*(H_t + (J/0.98)@y + ...) ; y = tanh(dt*H)
    via the ACT scale argument. The rescaling keeps (J/0.98) entries in
    fp16's normal range.
  - Split-precision fp16 matmuls (the kernel is weight-load bound: fp32
    LDWEIGHTS is 2-pass, f32r 4x slower; fp16/bf16 get FWL; fp16's
    10-bit mantissa beats bf16's 8): J'' = (J/0.98).T split as A + B,
    A=fp16(J''), B=fp16(J''-A); ya = fp16(tanh(dt*H)) straight out of
    ACT (no extra cast on the critical chain). Readout is 4 small fp16
    matmuls from ya. Simulated end-to-end rel err vs fp32 reference:
    1.2e-3 (4.99e-3 if hw flushes fp16 subnormals in B) vs gate 2e-2.
  - Per-step PE stream (32 J matmuls + 4 readout matmuls), bank-major:
    all matmuls writing bank A first (c 0,1 then 2,3), then bank B's,
    then step t-1's readout. Bank A thus completes mid-stream and its
    stt (DVE 0.98*psum+bv) -> tanh chain hides under bank B's matmuls;
    bank B's chain hides under the readout + next step's bank-A work.
  - bv outer products Bmat x v_t are staged every LBV=8 steps straight
    into a double-buffered PSUM bank (4 fp16 matmuls, strided dst; the
    stt reads bv directly from PSUM). vel is loaded once, fp16, as a
    [1, T*B_LOC] row. No per-block DMA, no DVE copies.
"""

import math
import os
import sys

import numpy as np

sys.path.insert(0, "/opt/trn_rl_repo")

DT = 0.02
DECAY = 1.0 - DT          # 0.98
HIDDEN = 512
BATCH = 128
T_FULL = 1024
N_CORES = 8
B_LOC = BATCH // N_CORES  # 16
CB = HIDDEN // 128        # 4 hidden chunks


def build_nc(T=T_FULL, lbv=8, ro_bank=512):
    import concourse.bass as bass
    import concourse.tile as tile
    from concourse import bacc, mybir

    f32 = mybir.dt.float32
    f16 = mybir.dt.float16
    nc = bacc.Bacc()

    ja_h = nc.declare_dram_parameter("JA", [HIDDEN, HIDDEN], f16, isOutput=False)
    jb_h = nc.declare_dram_parameter("JB", [HIDDEN, HIDDEN], f16, isOutput=False)
    bmr_h = nc.declare_dram_parameter("bmr", [1, HIDDEN], f16, isOutput=False)
    wrt_h = nc.declare_dram_parameter("wrt", [128, CB], f16, isOutput=False)
    velt_h = nc.declare_dram_parameter("velT", [T, B_LOC], f16, isOutput=False)
    out_h = nc.declare_dram_parameter("out", [B_LOC, T], f32, isOutput=True)

    n_ro = (T + ro_bank - 1) // ro_bank

    with tile.TileContext(nc) as tc:
        with (
            tc.tile_pool(name="singles", bufs=1) as singles,
            tc.tile_pool(name="yp", bufs=2) as yp,
            tc.tile_pool(name="velp", bufs=2) as velp,
            tc.tile_pool(name="bvs", bufs=2) as bvs,
            tc.tile_pool(name="psum", bufs=1, space="PSUM") as pp,
        ):
            # ---- weights staging ----
            jta = singles.tile([128, CB, HIDDEN], f16, tag="jta")
            nc.sync.dma_start(out=jta, in_=ja_h.rearrange("(c p) i -> p c i", p=128))
            jtb = singles.tile([128, CB, HIDDEN], f16, tag="jtb")
            nc.sync.dma_start(out=jtb, in_=jb_h.rearrange("(c p) i -> p c i", p=128))
            bmr = singles.tile([1, HIDDEN], f16, tag="bmr")  # Bmat as a row
            nc.sync.dma_start(out=bmr, in_=bmr_h[:, :])
            # whole vel sequence, fp16, on one partition (T*B_LOC*2B = 32KB)
            velb = singles.tile([1, T * B_LOC], f16, tag="velB")
            nc.sync.dma_start(
                out=velb, in_=velt_h.rearrange("t b -> (t b)").unsqueeze(0)
            )
            wrt = singles.tile([128, CB], f16, tag="wrt")
            nc.sync.dma_start(out=wrt, in_=wrt_h[:, :])

            # zero lhsT/rhs used to clear+claim the H PSUM banks (start=True)
            zrow = singles.tile([1, 128], f32, tag="zrow")
            nc.vector.memset(zrow, 0.0)

            psum_z = [
                pp.tile([128, 2 * B_LOC], f32, tag=f"z{h}", name=f"psum_z{h}")
                for h in range(2)
            ]
            psum_ro = [
                pp.tile([B_LOC, ro_bank], f32, tag=f"ro{i}", name=f"psum_ro{i}")
                for i in range(n_ro)
            ]
            # junk PSUM target for "absorber" matmuls: each absorber consumes a
            # single fresh semaphore tick (DMA completion etc.) so that real
            # matmuls never need more than ONE sync wait (the self-loading
            # matmul's LDWEIGHTS slice has a single wait slot).
            pjunk = pp.tile([1, 8], f32, tag="junk", name="psum_junk")

            def absorb(src_1el):
                nc.tensor.matmul(
                    out=pjunk[0:1, 0:1],
                    lhsT=src_1el,
                    rhs=src_1el,
                    start=True,
                    stop=True,
                    skip_group_check=True,
                )

            for h in range(2):
                nc.tensor.matmul(
                    out=psum_z[h],
                    lhsT=zrow[0:1, 0:128],
                    rhs=zrow[0:1, 0 : 2 * B_LOC],
                    start=True,
                    stop=True,
                    skip_group_check=True,
                )

            # soak up the weight-staging DMA completions one at a time
            absorb(jta[0:1, 0, 0:1])
            absorb(jtb[0:1, 0, 0:1])
            absorb(wrt[0:1, 0:1])
            absorb(bmr[0:1, 0:1])
            absorb(velb[0:1, 0:1])

            ya_prev = yp.tile([128, CB, B_LOC], f16, tag="ya")
            nc.vector.memset(ya_prev.rearrange("p c b -> p (c b)"), 0.0)

            def jmm(q, c, rhs):
                col = B_LOC * (q % 2)
                for jt_w in (jta, jtb):
                    nc.tensor.matmul(
                        out=psum_z[q // 2][:, col : col + B_LOC],
                        lhsT=jt_w[:, c, 128 * q : 128 * (q + 1)],
                        rhs=rhs,
                        start=False,
                        stop=False,
                        skip_group_check=True,
                    )

            def emit_ro(tm, ya):
                # psum_ro[:, tm] = sum_c ya_c.T @ W_ro_c.T  (4 fp16 matmuls)
                rb, rc = tm // ro_bank, tm % ro_bank
                for c in range(CB):
                    nc.tensor.matmul(
                        out=psum_ro[rb][0:B_LOC, rc : rc + 1],
                        lhsT=ya[:, c, :],
                        rhs=wrt[:, c : c + 1],
                        start=(c == 0),
                        stop=(c == CB - 1),
                        skip_group_check=True,
                    )
                if rc == ro_bank - 1 or tm == T - 1:
                    out_sb = velp.tile([B_LOC, ro_bank], f32, tag="osb", name="out_sb")
                    nc.vector.tensor_copy(out_sb[:, 0 : rc + 1], psum_ro[rb][:, 0 : rc + 1])
                    nc.sync.dma_start(
                        out=out_h[:, rb * ro_bank : rb * ro_bank + rc + 1],
                        in_=out_sb[:, 0 : rc + 1],
                    )

            bvp = None
            for t in range(T):
                j = t % lbv
                if j == 0:
                    # stage LBV steps of Bmat x v outer products straight into
                    # a PSUM bank (fp16 matmuls, strided dst; the stt reads bv
                    # from PSUM — no DVE copies, no per-block DMA):
                    #   bvp[p, (t, c, b)] = bmr[128c+p] * v[t, b]
                    pbv = pp.tile([128, lbv, CB, B_LOC], f32, tag="pbv", bufs=2, name="psum_bv")
                    for c in range(CB):
                        nc.tensor.matmul(
                            out=pbv[:, :, c, :],
                            lhsT=bmr[0:1, 128 * c : 128 * (c + 1)],
                            rhs=velb[0:1, t * B_LOC : (t + lbv) * B_LOC],
                            start=True,
                            stop=True,
                            skip_group_check=True,
                        )
                    # one copy to SBUF per block: the stt may read only one
                    # PSUM input (psum_z), so bv must come from SBUF
                    bvp = bvs.tile([128, lbv, CB, B_LOC], f32, tag="bvs")
                    nc.vector.tensor_copy(
                        bvp.rearrange("p t c b -> p (t c b)"),
                        pbv.rearrange("p t c b -> p (t c b)"),
                    )

                ya_new = yp.tile([128, CB, B_LOC], f16, tag="ya")

                # Stream order [A-c01][B-c01][A-c23][B-c23]: the early-gated
                # c01 sections (needing only tanh_A(t-1)) run first; both
                # banks' c23 sections run back-to-back once tanh_B(t-1) lands,
                # so bank B completes right after bank A and both stt->tanh
                # chains overlap the readout + next step's early sections.
                def half_tail(h):
                    # psum_h = 0.98*psum_h + Bmat x v_t  (chunks 2h, 2h+1)
                    nc.vector.scalar_tensor_tensor(
                        out=psum_z[h],
                        in0=psum_z[h],
                        scalar=float(DECAY),
                        in1=bvp[:, j, 2 * h : 2 * h + 2, :].rearrange(
                            "p c b -> p (c b)"
                        ),
                        op0=mybir.AluOpType.mult,
                        op1=mybir.AluOpType.add,
                    )
                    # ya = fp16(tanh(dt*H))  (ACT, critical)
                    nc.scalar.activation(
                        out=ya_new[:, 2 * h : 2 * h + 2, :].rearrange("p c b -> p (c b)"),
                        in_=psum_z[h],
                        func=mybir.ActivationFunctionType.Tanh,
                        scale=float(DT),
                    )

                for h in range(2):
                    for c in (0, 1):
                        for q in (2 * h, 2 * h + 1):
                            jmm(q, c, ya_prev[:, c, :])
                for h in range(2):
                    for c in (2, 3):
                        for q in (2 * h, 2 * h + 1):
                            jmm(q, c, ya_prev[:, c, :])
                    half_tail(h)

                # readout for step t-1 at stream end: its inputs are long
                # ready, and it gives the PE tail-window work while step t's
                # tanh chain completes
                if t > 0:
                    emit_ro(t - 1, ya_prev)

                ya_prev = ya_new

            emit_ro(T - 1, ya_prev)

    nc.compile()
    return nc


_NC_CACHE = {}


def _get_nc(**kw):
    key = tuple(sorted(kw.items()))
    if key not in _NC_CACHE:
        _NC_CACHE[key] = build_nc(**kw)
    return _NC_CACHE[key]


def make_in_maps(vel, J, Bmat, W_ro):
    vel = np.asarray(vel, dtype=np.float32)[:, :, 0]          # [B, T]
    J = np.asarray(J, dtype=np.float32)
    Bmat = np.asarray(Bmat, dtype=np.float32)
    W_ro = np.asarray(W_ro, dtype=np.float32)

    jt = np.ascontiguousarray((J / np.float32(DECAY)).T)       # [512, 512]
    ja = jt.astype(np.float16)
    jb = (jt - ja.astype(np.float32)).astype(np.float16)
    bmr = Bmat[:, 0].reshape(1, HIDDEN).astype(np.float16)    # [1, 512]
    wrt = W_ro[0].reshape(CB, 128).T.astype(np.float16)        # [128, 4]
    return [
        {
            "JA": ja,
            "JB": jb,
            "bmr": np.ascontiguousarray(bmr),
            "wrt": np.ascontiguousarray(wrt),
            "velT": np.ascontiguousarray(vel[c * B_LOC : (c + 1) * B_LOC].T.astype(np.float16)),
        }
        for c in range(N_CORES)
    ]


def kernel(vel, J, Bmat, W_ro, _trace=False, **build_kw):
    from concourse.bass_utils import run_bass_kernel_spmd

    nc = _get_nc(**build_kw)
    in_maps = make_in_maps(vel, J, Bmat, W_ro)
    res = run_bass_kernel_spmd(
        nc, in_maps, list(range(N_CORES)), trace=_trace
    )
    out = np.concatenate([r["out"] for r in res.results], axis=0)
    out = out[:, :, None].astype(np.float32)
    if _trace:
        kernel.last_results = res
    return out


kernel.last_results = None


# revision 24
# speedup vs baseline: 1.3670x; 1.3670x over previous
"""CTRNN forward kernel for Trainium2 (8 NeuronCores, data-parallel over batch).

Reference computation (per step t, dt=0.02):
    h = h*(1-dt) + dt*(tanh(h) @ J.T + v_t @ Bmat.T)
    out_t = tanh(h) @ W_ro.T

Device mapping (per core, B_LOC=16 of the 128 batch rows):
  - Layout: [hidden-on-partitions x batch-on-free] ("hT"); hidden 512 =
    4 chunks of 128 partitions; chunk q row p = hidden index 128*q+p.
  - State is rescaled: PSUM holds H = h/dt in 2 half banks [128, 32]
    (bank Hf holds chunks 2Hf, 2Hf+1 at cols 0:16 / 16:32), never
    evicted:  H_{t+1} = 0.98*(H_t + (J/0.98)@y + ...) ; y = tanh(dt*H)
    via the ACT scale argument. The rescaling keeps (J/0.98) entries in
    fp16's normal range.
  - Split-precision fp16 matmuls (the kernel is weight-load bound: fp32
    LDWEIGHTS is 2-pass, f32r 4x slower; fp16/bf16 get FWL; fp16's
    10-bit mantissa beats bf16's 8): J'' = (J/0.98).T split as A + B,
    A=fp16(J''), B=fp16(J''-A); ya = fp16(tanh(dt*H)) straight out of
    ACT (no extra cast on the critical chain). Readout is 4 small fp16
    matmuls from ya. Simulated end-to-end rel err vs fp32 reference:
    1.2e-3 (4.99e-3 if hw flushes fp16 subnormals in B) vs gate 2e-2.
  - Per-step PE stream (32 J matmuls + 4 readout matmuls), bank-major:
    all matmuls writing bank A first (c 0,1 then 2,3), then bank B's,
    then step t-1's readout. Bank A thus completes mid-stream and its
    stt (DVE 0.98*psum+bv) -> tanh chain hides under bank B's matmuls;
    bank B's chain hides under the readout + next step's bank-A work.
  - bv outer products Bmat x v_t are staged every LBV=8 steps straight
    into a double-buffered PSUM bank (4 fp16 matmuls, strided dst; the
    stt reads bv directly from PSUM). vel is loaded once, fp16, as a
    [1, T*B_LOC] row. No per-block DMA, no DVE copies.
"""

import math
import os
import sys

import numpy as np

sys.path.insert(0, "/opt/trn_rl_repo")

DT = 0.02
DECAY = 1.0 - DT          # 0.98
HIDDEN = 512
BATCH = 128
T_FULL = 1024
N_CORES = 8
B_LOC = BATCH // N_CORES  # 16
CB = HIDDEN // 128        # 4 hidden chunks


def build_nc(T=T_FULL, lbv=8, ro_bank=512):
    import concourse.bass as bass
    import concourse.tile as tile
    from concourse import bacc, mybir

    f32 = mybir.dt.float32
    f16 = mybir.dt.float16
    nc = bacc.Bacc()

    ja_h = nc.declare_dram_parameter("JA", [HIDDEN, HIDDEN], f16, isOutput=False)
    jb_h = nc.declare_dram_parameter("JB", [HIDDEN, HIDDEN], f16, isOutput=False)
    bmr_h = nc.declare_dram_parameter("bmr", [1, HIDDEN], f16, isOutput=False)
    wrt_h = nc.declare_dram_parameter("wrt", [128, CB], f16, isOutput=False)
    velt_h = nc.declare_dram_parameter("velT", [T, B_LOC], f16, isOutput=False)
    out_h = nc.declare_dram_parameter("out", [B_LOC, T], f32, isOutput=True)

    n_ro = (T + ro_bank - 1) // ro_bank

    with tile.TileContext(nc) as tc:
        with (
            tc.tile_pool(name="singles", bufs=1) as singles,
            tc.tile_pool(name="yp", bufs=2) as yp,
            tc.tile_pool(name="velp", bufs=2) as velp,
            tc.tile_pool(name="bvs", bufs=2) as bvs,
            tc.tile_pool(name="psum", bufs=1, space="PSUM") as pp,
        ):
            # ---- weights staging ----
            jta = singles.tile([128, CB, HIDDEN], f16, tag="jta")
            nc.sync.dma_start(out=jta, in_=ja_h.rearrange("(c p) i -> p c i", p=128))
            jtb = singles.tile([128, CB, HIDDEN], f16, tag="jtb")
            nc.sync.dma_start(out=jtb, in_=jb_h.rearrange("(c p) i -> p c i", p=128))
            bmr = singles.tile([1, HIDDEN], f16, tag="bmr")  # Bmat as a row
            nc.sync.dma_start(out=bmr, in_=bmr_h[:, :])
            # whole vel sequence, fp16, on one partition (T*B_LOC*2B = 32KB)
            velb = singles.tile([1, T * B_LOC], f16, tag="velB")
            nc.sync.dma_start(
                out=velb, in_=velt_h.rearrange("t b -> (t b)").unsqueeze(0)
            )
            wrt = singles.tile([128, CB], f16, tag="wrt")
            nc.sync.dma_start(out=wrt, in_=wrt_h[:, :])

            # zero lhsT/rhs used to clear+claim the H PSUM banks (start=True)
            zrow = singles.tile([1, 128], f32, tag="zrow")
            nc.vector.memset(zrow, 0.0)

            psum_z = [
                pp.tile([128, 2 * B_LOC], f32, tag=f"z{h}", name=f"psum_z{h}")
                for h in range(2)
            ]
            psum_ro = [
                pp.tile([B_LOC, ro_bank], f32, tag=f"ro{i}", name=f"psum_ro{i}")
                for i in range(n_ro)
            ]
            # junk PSUM target for "absorber" matmuls: each absorber consumes a
            # single fresh semaphore tick (DMA completion etc.) so that real
            # matmuls never need more than ONE sync wait (the self-loading
            # matmul's LDWEIGHTS slice has a single wait slot).
            pjunk = pp.tile([1, 8], f32, tag="junk", name="psum_junk")

            def absorb(src_1el):
                nc.tensor.matmul(
                    out=pjunk[0:1, 0:1],
                    lhsT=src_1el,
                    rhs=src_1el,
                    start=True,
                    stop=True,
                    skip_group_check=True,
                )

            for h in range(2):
                nc.tensor.matmul(
                    out=psum_z[h],
                    lhsT=zrow[0:1, 0:128],
                    rhs=zrow[0:1, 0 : 2 * B_LOC],
                    start=True,
                    stop=True,
                    skip_group_check=True,
                )

            # soak up the weight-staging DMA completions one at a time
            absorb(jta[0:1, 0, 0:1])
            absorb(jtb[0:1, 0, 0:1])
            absorb(wrt[0:1, 0:1])
            absorb(bmr[0:1, 0:1])
            absorb(velb[0:1, 0:1])

            ya_prev = yp.tile([128, CB, B_LOC], f16, tag="ya")
            nc.vector.memset(ya_prev.rearrange("p c b -> p (c b)"), 0.0)

            def jmm(q, c, rhs):
                # A-only: the fp16 residual B contributes ~1.6e-3 end-to-end
                # (sim: 5.1e-3 vs 3.5e-3 with B, gate 2e-2) and dropping it
                # halves the weight-load-bound matmul stream
                col = B_LOC * (q % 2)
                for jt_w in (jta,):
                    nc.tensor.matmul(
                        out=psum_z[q // 2][:, col : col + B_LOC],
                        lhsT=jt_w[:, c, 128 * q : 128 * (q + 1)],
                        rhs=rhs,
                        start=False,
                        stop=False,
                        skip_group_check=True,
                    )

            def emit_ro(tm, ya):
                # psum_ro[:, tm] = sum_c ya_c.T @ W_ro_c.T  (4 fp16 matmuls)
                rb, rc = tm // ro_bank, tm % ro_bank
                for c in range(CB):
                    nc.tensor.matmul(
                        out=psum_ro[rb][0:B_LOC, rc : rc + 1],
                        lhsT=ya[:, c, :],
                        rhs=wrt[:, c : c + 1],
                        start=(c == 0),
                        stop=(c == CB - 1),
                        skip_group_check=True,
                    )
                if rc == ro_bank - 1 or tm == T - 1:
                    out_sb = velp.tile([B_LOC, ro_bank], f32, tag="osb", name="out_sb")
                    nc.vector.tensor_copy(out_sb[:, 0 : rc + 1], psum_ro[rb][:, 0 : rc + 1])
                    nc.sync.dma_start(
                        out=out_h[:, rb * ro_bank : rb * ro_bank + rc + 1],
                        in_=out_sb[:, 0 : rc + 1],
                    )

            bvp = None
            for t in range(T):
                j = t % lbv
                if j == 0:
                    # stage LBV steps of Bmat x v outer products straight into
                    # a PSUM bank (fp16 matmuls, strided dst; the stt reads bv
                    # from PSUM — no DVE copies, no per-block DMA):
                    #   bvp[p, (t, c, b)] = bmr[128c+p] * v[t, b]
                    pbv = pp.tile([128, lbv, CB, B_LOC], f32, tag="pbv", bufs=2, name="psum_bv")
                    for c in range(CB):
                        nc.tensor.matmul(
                            out=pbv[:, :, c, :],
                            lhsT=bmr[0:1, 128 * c : 128 * (c + 1)],
                            rhs=velb[0:1, t * B_LOC : (t + lbv) * B_LOC],
                            start=True,
                            stop=True,
                            skip_group_check=True,
                        )
                    # one copy to SBUF per block: the stt may read only one
                    # PSUM input (psum_z), so bv must come from SBUF
                    bvp = bvs.tile([128, lbv, CB, B_LOC], f32, tag="bvs")
                    nc.vector.tensor_copy(
                        bvp.rearrange("p t c b -> p (t c b)"),
                        pbv.rearrange("p t c b -> p (t c b)"),
                    )

                ya_new = yp.tile([128, CB, B_LOC], f16, tag="ya")

                # Bank-major stream: ALL matmuls writing bank h first, so bank
                # A completes ~1/2 into the stream and its stt->tanh chain
                # hides under bank B's matmuls; bank B's chain hides under the
                # readout + next step's bank-A matmuls.
                for h in range(2):
                    for c in (0, 1):
                        for q in (2 * h, 2 * h + 1):
                            jmm(q, c, ya_prev[:, c, :])
                    for c in (2, 3):
                        for q in (2 * h, 2 * h + 1):
                            jmm(q, c, ya_prev[:, c, :])
                    # psum_h = 0.98*psum_h + Bmat x v_t  (chunks 2h, 2h+1)
                    nc.vector.scalar_tensor_tensor(
                        out=psum_z[h],
                        in0=psum_z[h],
                        scalar=float(DECAY),
                        in1=bvp[:, j, 2 * h : 2 * h + 2, :].rearrange(
                            "p c b -> p (c b)"
                        ),
                        op0=mybir.AluOpType.mult,
                        op1=mybir.AluOpType.add,
                    )
                    # ya = fp16(tanh(dt*H))  (ACT, critical)
                    nc.scalar.activation(
                        out=ya_new[:, 2 * h : 2 * h + 2, :].rearrange("p c b -> p (c b)"),
                        in_=psum_z[h],
                        func=mybir.ActivationFunctionType.Tanh,
                        scale=float(DT),
                    )

                # readout for step t-1 at stream end: its inputs are long
                # ready, and it gives the PE tail-window work while step t's
                # tanh chain completes
                if t > 0:
                    emit_ro(t - 1, ya_prev)

                ya_prev = ya_new

            emit_ro(T - 1, ya_prev)

    nc.compile()
    return nc


_NC_CACHE = {}


def _get_nc(**kw):
    key = tuple(sorted(kw.items()))
    if key not in _NC_CACHE:
        _NC_CACHE[key] = build_nc(**kw)
    return _NC_CACHE[key]


def make_in_maps(vel, J, Bmat, W_ro):
    vel = np.asarray(vel, dtype=np.float32)[:, :, 0]          # [B, T]
    J = np.asarray(J, dtype=np.float32)
    Bmat = np.asarray(Bmat, dtype=np.float32)
    W_ro = np.asarray(W_ro, dtype=np.float32)

    jt = np.ascontiguousarray((J / np.float32(DECAY)).T)       # [512, 512]
    ja = jt.astype(np.float16)
    jb = (jt - ja.astype(np.float32)).astype(np.float16)
    bmr = Bmat[:, 0].reshape(1, HIDDEN).astype(np.float16)    # [1, 512]
    wrt = W_ro[0].reshape(CB, 128).T.astype(np.float16)        # [128, 4]
    return [
        {
            "JA": ja,
            "JB": jb,
            "bmr": np.ascontiguousarray(bmr),
            "wrt": np.ascontiguousarray(wrt),
            "velT": np.ascontiguousarray(vel[c * B_LOC : (c + 1) * B_LOC].T.astype(np.float16)),
        }
        for c in range(N_CORES)
    ]


def kernel(vel, J, Bmat, W_ro, _trace=False, **build_kw):
    from concourse.bass_utils import run_bass_kernel_spmd

    nc = _get_nc(**build_kw)
    in_maps = make_in_maps(vel, J, Bmat, W_ro)
    res = run_bass_kernel_spmd(
        nc, in_maps, list(range(N_CORES)), trace=_trace
    )
    out = np.concatenate([r["out"] for r in res.results], axis=0)
    out = out[:, :, None].astype(np.float32)
    if _trace:
        kernel.last_results = res
    return out


kernel.last_results = None
